# revision 41
# baseline (speedup 1.0000x reference)
"""Trainium2 Bass kernel for nn_MultiHeadAttention_34162169872901 — v3.

MultiHeadAttention (B=4, S=2048, d_model=512, 8 heads, d_k=64) with a
relative-position bias table (511 entries, clamp +-255) and an all-ones mask.

Sharding (8 NeuronCores): core c handles batch b = c//2 and 4 of the 8 heads
(c%2 selects the head half); host sums the two partial outputs per batch.

v3 changes vs v2 (222.8us):
  - Input DMAs issued from the tensor/scalar sequencers (their preamble is
    ~6us shorter than sync's) and xk/xv split in wave-halves, so K-projection
    starts ~7us earlier and phase-A DMA stalls shrink.
  - PE p-state warm-up matmuls during the initial DMA wait.
  - O-projection packed to 128-deep contraction: the odd head's normalized
    ctx is DMA-shifted into partitions 64..127 of a paired cx tile, halving
    O-proj matmul count.
  - QK matmuls emitted ah-major so PE tile_position switches halve and the
    first exp of each group starts one matmul earlier.
"""

import sys
import types

import numpy as np

B = 4
S = 2048
D = 512
NHEAD = 8
DK = 64
NCORES = 8
MAX_REL = 255
NKT = S // 128   # 16 k-tiles
NU = S // 512    # 4 q-units


def _install_axon_hooks():
    try:
        import antenv
    except ImportError:
        return
    try:
        from antenv.axon_hooks import get_axon_ntff_profile_hook  # noqa: F401
        return
    except ImportError:
        pass
    hook = None
    try:
        from trn_agent_boot.trn_boot import _ntff_profile_via_ctypes
        hook = _ntff_profile_via_ctypes("/opt/axon/libaxon_pjrt.so")
    except Exception:
        hook = None
    m = types.ModuleType("antenv.axon_hooks")
    m.get_axon_ntff_profile_hook = lambda: hook
    m.set_axon_ntff_profile_hook = lambda h: None
    sys.modules["antenv.axon_hooks"] = m
    antenv.axon_hooks = m


_install_axon_hooks()

import concourse.bass as bass  # noqa: E402
import concourse.bacc as bacc  # noqa: E402
import concourse.mybir as mybir  # noqa: E402
from concourse import tile  # noqa: E402
from concourse.bass_utils import run_bass_kernel_spmd  # noqa: E402
from concourse.vector_clock import ScopedClock as _ScopedClock  # noqa: E402

f32 = mybir.dt.float32
f32r = mybir.dt.float32r
f16 = mybir.dt.float16
AF = mybir.ActivationFunctionType


def _patched_drain_and_barrier(self, tick_clock, wait_clock):
    # walrus in this container rejects >2 sem waits on one instruction; emit
    # the tail-drain waits as standalone wait instructions instead.
    nc = self.nc
    dummy = mybir.InstNoOp(name="drain-wait-probe", engine=mybir.EngineType.SP)
    wait_clock.add_sem_waits(dummy, _ScopedClock({None: tick_clock.global_clock}))
    handles = {h.name: h for h in self.sems.allocated().values()}
    si = dummy.sync_info
    for w in (si.on_wait if si is not None else []):
        nc.sync.wait_ge(handles[w.ant_name], w.wait_value)
    nc.sync.drain()
    nc.all_engine_barrier()
    popped = nc._tile_sem_poison_stack.pop()
    assert popped is self._sem_poison
    nc.clear_and_free_semaphores(list(self.sems.allocated().values()))
    nc.all_engine_barrier()


tile.TileContext._drain_and_barrier = _patched_drain_and_barrier


def _delta(t, u):
    return 128 * t - 512 * u


def _cls(t, u):
    d = _delta(t, u)
    if d <= -384:
        return 1  # whole block clamps to table[0]
    if d >= 768:
        return 2  # whole block clamps to table[510]
    return 0      # in-band: needs the Toeplitz block


def _didx(t, u):
    return (_delta(t, u) + 256) // 128  # 0..7 for in-band blocks


def build_program():
    import os
    kdebug = os.environ.get("KDEBUG", "0") == "1"
    nc = bacc.Bacc()

    xqT = nc.declare_dram_parameter("xqT", [D, S], f16, isOutput=False)
    xkT = nc.declare_dram_parameter("xkT", [D, S], f16, isOutput=False)
    xvT = nc.declare_dram_parameter("xvT", [D, S], f16, isOutput=False)
    wq = nc.declare_dram_parameter("wq", [128, 4, 256], f16, isOutput=False)
    wk = nc.declare_dram_parameter("wk", [128, 4, 256], f16, isOutput=False)
    wv = nc.declare_dram_parameter("wv", [128, 4, 256], f16, isOutput=False)
    wo = nc.declare_dram_parameter("wo", [128, 2, 512], f16, isOutput=False)
    ebd = nc.declare_dram_parameter("eb", [128, 4, 8, 512], f16, isOutput=False)
    cbd = nc.declare_dram_parameter("cb", [128, 4, 3], f32, isOutput=False)
    outd = nc.declare_dram_parameter("out", [S, D], f32, isOutput=True)

    with tile.TileContext(nc) as tc:
        with (
            tc.tile_pool(name="sb", bufs=1) as pool,
            tc.tile_pool(name="xt", bufs=1) as xpool,
            tc.tile_pool(name="pt", bufs=6) as ppool,
            tc.tile_pool(name="cxs", bufs=1) as cpool,
            tc.tile_pool(name="obp", bufs=4) as opool,
            tc.tile_pool(name="scp", bufs=2, space="PSUM") as scp,
            tc.tile_pool(name="cxp", bufs=2, space="PSUM") as cxp,
            tc.tile_pool(name="msc", bufs=2, space="PSUM") as msc,
        ):
            # ---- persistent SBUF tiles -------------------------------------
            wq_sb = pool.tile([128, 4, 256], f16, tag="wq")
            wk_sb = pool.tile([128, 4, 256], f16, tag="wk")
            wv_sb = pool.tile([128, 4, 256], f16, tag="wv")
            wo_sb = pool.tile([128, 2, 512], f16, tag="wo")
            eb_sb = pool.tile([128, 4, 8, 512], f16, tag="eb")
            cb_sb = pool.tile([128, 4, 3], f32, tag="cb")
            qt_sb = pool.tile([128, 2, S], f16, tag="qt")
            kt_sb = pool.tile([128, 2, S], f16, tag="kt")
            v_sb = pool.tile([128, NKT, 4 * 65], f16, tag="v")
            onesb = pool.tile([128, 64], f16, tag="ones")
            warm = pool.tile([128, 16], f32, tag="warm")
            wtile = pool.tile([128, 512], f16, tag="wtile")

            # ---- DMAs, priority-ordered ------------------------------------
            # gpsimd issues the K path first in wave halves; gating schemes
            # and engine shuffles all measured worse (slow 0.4us/issue
            # sequencer trains + ring variance), so keep it simple.
            xks = [xpool.tile([128, S], f16, tag="xk", bufs=4, name=f"xk{ct}")
                   for ct in range(4)]
            nc.gpsimd.dma_start(wk_sb[:], wk[:])
            for half in range(2):
                for ct in range(4):
                    nc.gpsimd.dma_start(
                        xks[ct][:, half * 1024:(half + 1) * 1024],
                        xkT[ct * 128:(ct + 1) * 128,
                            half * 1024:(half + 1) * 1024])
            nc.gpsimd.dma_start(wo_sb[:], wo[:])
            # vector's user stream starts early (~7.3us); memset there so
            # the PE warm-up isn't gated on slow dma_start issues.
            nc.vector.memset(wtile[:], 0.0)
            # scalar queue: V path in halves, plus the tiny cb table.
            xvs = [xpool.tile([128, S], f16, tag="xv", bufs=4, name=f"xv{ct}")
                   for ct in range(4)]
            nc.scalar.dma_start(wv_sb[:], wv[:])
            nc.scalar.dma_start(cb_sb[:], cbd[:])
            for half in range(2):
                for ct in range(4):
                    nc.scalar.dma_start(
                        xvs[ct][:, half * 1024:(half + 1) * 1024],
                        xvT[ct * 128:(ct + 1) * 128,
                            half * 1024:(half + 1) * 1024])
            # sync queue: Q path in u-chunks (u0 first), then eb pieces in
            # first-use order (u0 in-band blocks need didx pairs 2,4,6).
            nc.sync.dma_start(wq_sb[:], wq[:])
            xqs = [xpool.tile([128, S], f16, tag="xq", bufs=4, name=f"xq{ct}")
                   for ct in range(4)]
            for u in range(NU):
                for ct in range(4):
                    nc.sync.dma_start(
                        xqs[ct][:, u * 512:(u + 1) * 512],
                        xqT[ct * 128:(ct + 1) * 128, u * 512:(u + 1) * 512])
            for d0 in (2, 4, 6, 0):
                nc.sync.dma_start(eb_sb[:, :, d0:d0 + 2, :],
                                  ebd[:, :, d0:d0 + 2, :])

            # PE p-state warm-up while the first DMAs land (bridges until
            # wk/xk wave 0 arrive so K-proj starts at full clock).
            wpsum = msc.tile([128, 512], f32, tag="ms", name="warmpm")
            for _ in range(12):
                nc.tensor.matmul(wpsum[0:16, :], lhsT=wtile[:, 0:16],
                                 rhs=wtile[:], start=True, stop=True)

            nc.vector.memset(onesb[:], 1.0)
            # preload the exp table while DMAs stream in
            nc.vector.memset(warm[:], 0.0)
            nc.scalar.activation(warm[:], warm[:], AF.Exp, bias=0.0, scale=1.0)

            # ---- phase A: K-proj, V-proj, Q-proj(u0) -----------------------
            # K-projection: 2 waves of (2 sc-chunks x 2 hp) using scp tiles.
            for wave in range(2):
                pks = {hp: scp.tile([128, 1024], f32, tag="sc", name=f"pk{wave}{hp}")
                       for hp in range(2)}
                for ct in range(4):
                    for hp in range(2):
                        for sc2 in range(2):
                            sc = wave * 2 + sc2
                            nc.tensor.matmul(
                                pks[hp][:, sc2 * 512:(sc2 + 1) * 512],
                                lhsT=wk_sb[:, ct, hp * 128:(hp + 1) * 128],
                                rhs=xks[ct][:, sc * 512:(sc + 1) * 512],
                                start=(ct == 0), stop=(ct == 3),
                            )
                for hp in range(2):
                    nc.vector.tensor_copy(
                        kt_sb[:, hp, wave * 1024:(wave + 1) * 1024], pks[hp][:])
            def emit_qproj(u):
                for hp in range(2):
                    pq = msc.tile([128, 512], f32, tag="ms", name=f"pq{u}{hp}")
                    for ct in range(4):
                        nc.tensor.matmul(
                            pq[:],
                            lhsT=wq_sb[:, ct, hp * 128:(hp + 1) * 128],
                            rhs=xqs[ct][:, u * 512:(u + 1) * 512],
                            start=(ct == 0), stop=(ct == 3),
                        )
                    nc.vector.tensor_copy(qt_sb[:, hp, u * 512:(u + 1) * 512],
                                          pq[:])

            # Q-proj u0 BEFORE V-proj: its qt copy is what gates the first
            # QK matmul, while V is not needed until the first AV a group
            # later — this removes the qt wait from the phase-B start.
            emit_qproj(0)

            # V-projection: 2 waves of 8 s-tiles; ones column FIRST per head.
            for wave in range(2):
                pvs = [scp.tile([128, 1024], f32, tag="sc", name=f"pv{wave}{i}")
                       for i in range(2)]
                # st outer / ct inner: accumulation chains sharing a PSUM bank
                # must run sequentially (first_mm clears the bank's
                # has_written bits, so interleaved starts drop contributions)
                for st8 in range(8):
                    st = wave * 8 + st8
                    for ct in range(4):
                        nc.tensor.matmul(
                            pvs[st8 // 4][:, (st8 % 4) * 256:(st8 % 4) * 256 + 256],
                            lhsT=xvs[ct][:, st * 128:(st + 1) * 128],
                            rhs=wv_sb[:, ct, :],
                            start=(ct == 0), stop=(ct == 3),
                        )
                for st8 in range(8):
                    st = wave * 8 + st8
                    vslice = v_sb[:, st, :].rearrange("p (h x) -> p h x", x=65)
                    nc.vector.memset(vslice[:, :, 64:65], 1.0)
                    nc.vector.tensor_copy(
                        vslice[:, :, 0:64],
                        pvs[st8 // 4][:, (st8 % 4) * 256:(st8 % 4) * 256 + 256]
                        .rearrange("p (h x) -> p h x", x=64),
                    )

            def dump(dst_row, src_ap, nrows, ncols):
                t_ = opool.tile([nrows, 512], f32, tag="dbg", bufs=8,
                                name=f"dbg{dst_row}")
                nc.vector.tensor_copy(t_[:, 0:ncols], src_ap)
                nc.sync.dma_start(outd[dst_row:dst_row + nrows, 0:ncols],
                                  t_[:, 0:ncols])

            if kdebug:
                dump(256, qt_sb[:, 0, 0:512], 128, 512)
                dump(384, kt_sb[:, 0, 0:512], 128, 512)
                dump(512, v_sb[:, 0, 0:260], 128, 260)

            # ---- phase B: 8 slots (u, hp), norm/O-proj deferred one slot ---
            slots = [(u, hp) for u in range(NU) for hp in range(2)]
            prev = None          # (u, hp, ctxp pair) awaiting norm
            cxp2 = {}            # (u, hp) -> [128, 512] paired ctx tile
            pend_oproj = None    # u awaiting O-projection

            def emit_norm_pre(pv):
                # DVE part of the deferred norm chain for slot pv
                u0_, hp_, ctxp_ = pv
                outs = []
                for ah in range(2):
                    ctxf = cpool.tile([65, 512], f32, tag="ctxf", bufs=4)
                    nc.vector.tensor_copy(ctxf[:], ctxp_[ah][:])
                    if kdebug and (u0_, hp_) == (0, 0):
                        nc.sync.dma_start(
                            outd[640 + 65 * ah:640 + 65 * ah + 65, :], ctxf[:])
                    lp0 = cpool.tile([1, 512], f32, tag="lp0", bufs=2)
                    nc.gpsimd.dma_start(lp0[:], ctxf[64:65, :])
                    linv = cpool.tile([1, 512], f32, tag="linv", bufs=2)
                    nc.vector.reciprocal_approx_fast(linv[:], lp0[:])
                    linvb = cpool.tile([1, 512], f16, tag="linvb", bufs=2)
                    nc.vector.tensor_scalar_mul(linvb[:], linv[:], 256.0)
                    outs.append((ctxf, linvb))
                return outs

            def emit_norm_post(pv, outs):
                # PE broadcast + DVE normalize for slot pv; both heads land
                # in one [128, 512] tile (odd head DMA-shifted to parts
                # 64..127) so O-proj contracts 128-deep per chunk.
                u0_, hp_, _ = pv
                cxp = cpool.tile([128, 512], f16, tag="cx2", bufs=3,
                                 name=f"cxp{hp_}")
                cxp2[(u0_, hp_)] = cxp
                for ah in range(2):
                    ctxf, linvb = outs[ah]
                    bc = msc.tile([128, 512], f32, tag="ms", name=f"bc{hp_}{ah}")
                    nc.tensor.matmul(bc[0:64, :], lhsT=onesb[0:1, :],
                                     rhs=linvb[:], start=True, stop=True)
                    if ah == 0:
                        nc.vector.tensor_mul(cxp[0:64, :], bc[0:64, :],
                                             ctxf[0:64, :])
                    else:
                        cxo = cpool.tile([64, 512], f16, tag="cxo", bufs=2)
                        nc.vector.tensor_mul(cxo[:], bc[0:64, :],
                                             ctxf[0:64, :])
                        nc.gpsimd.dma_start(cxp[64:128, :], cxo[:])

            def emit_oproj_qs(u0_, qs):
                po = msc.tile([128, 512], f32, tag="ms", name=f"po{qs}")
                for hp_ in range(2):
                    nc.tensor.matmul(
                        po[:],
                        lhsT=cxp2[(u0_, hp_)][:, qs * 128:(qs + 1) * 128],
                        rhs=wo_sb[:, hp_, :],
                        start=(hp_ == 0), stop=(hp_ == 1),
                    )
                ob = opool.tile([128, 512], f32, tag="ob")
                if u0_ == 3 and qs % 2 == 0:
                    nc.scalar.activation(ob[:], po[:], AF.Copy, bias=0.0,
                                         scale=1.0)
                else:
                    nc.vector.tensor_copy(ob[:], po[:])
                if not kdebug:
                    if u0_ == 3:
                        q_eng = (nc.sync, nc.gpsimd, nc.scalar, nc.sync)[qs]
                    else:
                        q_eng = nc.sync
                    q_eng.dma_start(
                        outd[u0_ * 512 + qs * 128:
                             u0_ * 512 + (qs + 1) * 128, :],
                        ob[:],
                    )

            # Flat group stream across all slots; the AV matmuls for group i
            # are emitted at step i+1 so the PE never waits on ACT in-order.
            def slot_gorder(u):
                return sorted(range(NKT // 2),
                              key=lambda g: (_cls(2 * g, u) == 0, g))

            items = []
            for (u, hp) in slots:
                for gi, g in enumerate(slot_gorder(u)):
                    items.append((u, hp, gi, g))

            state = {}           # live per-slot state keyed by (u, hp)
            pend_av = None       # (slotkey, g, src, avstart)
            norm_outs = None

            def emit_av(slotkey, g, src):
                # ti-outer/ah-inner: consecutive matmuls hit different PSUM
                # tiles so the next ldweights overlaps the current drain.
                st_ = state[slotkey]
                u_, hp_ = slotkey
                for ti in range(2):
                    t = 2 * g + ti
                    for ah in range(2):
                        lh = 2 * hp_ + ah
                        vsl = v_sb[:, t, :].rearrange(
                            "p (h x) -> p h x", x=65)[:, lh, :]
                        st_["nav"][ah] += 1
                        nc.tensor.matmul(
                            st_["ctxp"][ah][:],
                            lhsT=vsl,
                            rhs=src[ah][:, ti * 512:(ti + 1) * 512],
                            start=(st_["nav"][ah] == 1),
                            stop=(st_["nav"][ah] == NKT),
                        )

            for (u, hp, gi, g) in items:
                slotkey = (u, hp)
                cls = _cls(2 * g, u)
                sct = [scp.tile([128, 1024], f32, tag="sc", name=f"sct{i}")
                       for i in range(2)]
                for ti in range(2):
                    t = 2 * g + ti
                    for ah in range(2):
                        nc.tensor.matmul(
                            sct[ah][:, ti * 512:(ti + 1) * 512],
                            lhsT=kt_sb[ah * 64:(ah + 1) * 64, hp,
                                       t * 128:(t + 1) * 128],
                            rhs=qt_sb[ah * 64:(ah + 1) * 64, hp,
                                      u * 512:(u + 1) * 512],
                            start=True, stop=True,
                            tile_position=(ah * 64, 0),
                        )
                src = []
                for ah in range(2):
                    lh = 2 * hp + ah
                    pt = ppool.tile([128, 1024], f16, tag="pt", bufs=6)
                    nc.scalar.activation(
                        pt[:], sct[ah][:], AF.Exp,
                        bias=cb_sb[:, lh, cls:cls + 1], scale=1.0,
                    )
                    if cls == 0:
                        sr = ppool.tile([128, 1024], f16, tag="pt2", bufs=4)
                        d0 = _didx(2 * g, u)
                        nc.vector.tensor_mul(
                            sr[:],
                            pt[:],
                            eb_sb[:, lh, d0:d0 + 2, :]
                            .rearrange("p a b -> p (a b)"),
                        )
                    else:
                        sr = pt
                    src.append(sr)

                if kdebug and (u, hp, gi) == (0, 0, 0):
                    dump(0, src[0][:, 0:512], 128, 512)
                    dump(128, src[1][:, 0:512], 128, 512)

                # AV for the previous group in the flat stream
                if pend_av is not None:
                    emit_av(*pend_av)
                if gi == 0:
                    # slot start (prev slot's last AV now emitted): deferred
                    # norm for prev slot, then this slot's accumulators.
                    norm_outs = emit_norm_pre(prev) if prev is not None else None
                    state[slotkey] = {
                        "ctxp": [cxp.tile([65, 512], f32, tag="cp",
                                          name=f"ctxp{ah}") for ah in range(2)],
                        "nav": [0, 0],
                    }
                pend_av = (slotkey, g, src)

                # deferred work rides the ACT-bound slack
                if gi == 1 and norm_outs is not None:
                    emit_norm_post(prev, norm_outs)
                    if prev[1] == 1:       # hp==1 slot completed unit u-1
                        pend_oproj = prev[0]
                    norm_outs = None
                if gi in (3, 4, 5, 6) and pend_oproj is not None:
                    emit_oproj_qs(pend_oproj, gi - 3)
                    if gi == 6:
                        pend_oproj = None
                if gi == 7 and hp == 0 and u < 3:
                    emit_qproj(u + 1)
                if gi == 7:
                    prev = (u, hp, state[slotkey]["ctxp"])

            emit_av(*pend_av)

            # ---- tail: last slot's norm + O-proj ---------------------------
            norm_outs = emit_norm_pre(prev)
            emit_norm_post(prev, norm_outs)
            for qs in range(4):
                emit_oproj_qs(3, qs)
    nc.compile()
    return nc


_PROGRAM = None


def _get_program():
    global _PROGRAM
    if _PROGRAM is None:
        _PROGRAM = build_program()
    return _PROGRAM


# index table for the in-band Toeplitz bias blocks, shared across heads
_IDX = None


def _idx_table():
    global _IDX
    if _IDX is None:
        p = np.arange(128)[:, None]
        f = np.arange(512)[None, :]
        blocks = []
        for didx in range(8):
            delta = didx * 128 - 256
            blocks.append(np.clip(delta + p - f + 255, 0, 510))
        _IDX = np.stack(blocks, axis=0)  # [8, 128, 512]
    return _IDX


def kernel(**inputs):
    query = np.asarray(inputs["query"], dtype=np.float32)
    key = np.asarray(inputs["key"], dtype=np.float32)
    value = np.asarray(inputs["value"], dtype=np.float32)
    mask = np.asarray(inputs["mask"])
    Wq = np.asarray(inputs["Wq"], dtype=np.float32)
    Wk = np.asarray(inputs["Wk"], dtype=np.float32)
    Wv = np.asarray(inputs["Wv"], dtype=np.float32)
    Wo = np.asarray(inputs["Wo"], dtype=np.float32)
    bo = np.asarray(inputs["bo"], dtype=np.float32)
    rel_bias = np.asarray(inputs["rel_bias"], dtype=np.float32)

    if not np.all(mask != 0):
        raise NotImplementedError("kernel assumes an all-ones attention mask")

    nc = _get_program()
    idx = _idx_table()
    scale = np.float32(1.0 / np.sqrt(DK))

    in_maps = []
    for c in range(NCORES):
        b = c // 2
        hbase = (c % 2) * 4
        rows = slice(hbase * 64, (hbase + 4) * 64)

        wq_arr = np.ascontiguousarray(
            (Wq[rows, :] * scale).T.reshape(4, 128, 256).swapaxes(0, 1))
        wk_arr = np.ascontiguousarray(
            Wk[rows, :].T.reshape(4, 128, 256).swapaxes(0, 1))
        wv_arr = np.ascontiguousarray(
            Wv[rows, :].T.reshape(4, 128, 256).swapaxes(0, 1))

        wo_arr = np.empty((128, 2, 512), dtype=np.float32)
        eb_arr = np.empty((128, 4, 8, 512), dtype=np.float16)
        cb_arr = np.zeros((128, 4, 3), dtype=np.float32)
        for lh in range(4):
            g = hbase + lh
            hp, ah = divmod(lh, 2)
            wo_arr[ah * 64:(ah + 1) * 64, hp, :] = (
                Wo[:, g * 64:(g + 1) * 64].T * (1.0 / 256.0))
            tbl = rel_bias[g]
            eb_arr[:, lh, :, :] = np.exp(tbl)[idx].transpose(1, 0, 2)
            cb_arr[:, lh, 1] = tbl[0]
            cb_arr[:, lh, 2] = tbl[510]

        bf = np.float16
        in_maps.append({
            "xqT": np.ascontiguousarray(query[b].T).astype(bf),
            "xkT": np.ascontiguousarray(key[b].T).astype(bf),
            "xvT": np.ascontiguousarray(value[b].T).astype(bf),
            "wq": wq_arr.astype(bf), "wk": wk_arr.astype(bf),
            "wv": wv_arr.astype(bf), "wo": wo_arr.astype(bf),
            "eb": eb_arr, "cb": cb_arr,
        })

    res = run_bass_kernel_spmd(nc, in_maps, list(range(NCORES)), trace=False)

    out = np.zeros((B, S, D), dtype=np.float32)
    for c in range(NCORES):
        out[c // 2] += res.results[c]["out"]
    out += bo[None, None, :]
    return out



# revision 46
# speedup vs baseline: 1.1799x; 1.1799x over previous
"""Trainium2 Bass kernel for nn_MultiHeadAttention_34162169872901 — v3.

MultiHeadAttention (B=4, S=2048, d_model=512, 8 heads, d_k=64) with a
relative-position bias table (511 entries, clamp +-255) and an all-ones mask.

Sharding (8 NeuronCores): core c handles batch b = c//2 and 4 of the 8 heads
(c%2 selects the head half); host sums the two partial outputs per batch.

v3 changes vs v2 (222.8us):
  - Input DMAs issued from the tensor/scalar sequencers (their preamble is
    ~6us shorter than sync's) and xk/xv split in wave-halves, so K-projection
    starts ~7us earlier and phase-A DMA stalls shrink.
  - PE p-state warm-up matmuls during the initial DMA wait.
  - O-projection packed to 128-deep contraction: the odd head's normalized
    ctx is DMA-shifted into partitions 64..127 of a paired cx tile, halving
    O-proj matmul count.
  - QK matmuls emitted ah-major so PE tile_position switches halve and the
    first exp of each group starts one matmul earlier.
"""

import sys
import types

import numpy as np

B = 4
S = 2048
D = 512
NHEAD = 8
DK = 64
NCORES = 8
MAX_REL = 255
NKT = S // 128   # 16 k-tiles
NU = S // 512    # 4 q-units


def _install_axon_hooks():
    try:
        import antenv
    except ImportError:
        return
    try:
        from antenv.axon_hooks import get_axon_ntff_profile_hook  # noqa: F401
        return
    except ImportError:
        pass
    hook = None
    try:
        from trn_agent_boot.trn_boot import _ntff_profile_via_ctypes
        hook = _ntff_profile_via_ctypes("/opt/axon/libaxon_pjrt.so")
    except Exception:
        hook = None
    m = types.ModuleType("antenv.axon_hooks")
    m.get_axon_ntff_profile_hook = lambda: hook
    m.set_axon_ntff_profile_hook = lambda h: None
    sys.modules["antenv.axon_hooks"] = m
    antenv.axon_hooks = m


_install_axon_hooks()

import concourse.bass as bass  # noqa: E402
import concourse.bacc as bacc  # noqa: E402
import concourse.mybir as mybir  # noqa: E402
from concourse import tile  # noqa: E402
from concourse.bass_utils import run_bass_kernel_spmd  # noqa: E402
from concourse.vector_clock import ScopedClock as _ScopedClock  # noqa: E402

f32 = mybir.dt.float32
f32r = mybir.dt.float32r
f16 = mybir.dt.float16
AF = mybir.ActivationFunctionType


def _patched_drain_and_barrier(self, tick_clock, wait_clock):
    # walrus in this container rejects >2 sem waits on one instruction; emit
    # the tail-drain waits as standalone wait instructions instead.
    nc = self.nc
    dummy = mybir.InstNoOp(name="drain-wait-probe", engine=mybir.EngineType.SP)
    wait_clock.add_sem_waits(dummy, _ScopedClock({None: tick_clock.global_clock}))
    handles = {h.name: h for h in self.sems.allocated().values()}
    si = dummy.sync_info
    for w in (si.on_wait if si is not None else []):
        nc.sync.wait_ge(handles[w.ant_name], w.wait_value)
    nc.sync.drain()
    nc.all_engine_barrier()
    popped = nc._tile_sem_poison_stack.pop()
    assert popped is self._sem_poison
    nc.clear_and_free_semaphores(list(self.sems.allocated().values()))
    nc.all_engine_barrier()


tile.TileContext._drain_and_barrier = _patched_drain_and_barrier


def _delta(t, u):
    return 128 * t - 512 * u


def _cls(t, u):
    d = _delta(t, u)
    if d <= -384:
        return 1  # whole block clamps to table[0]
    if d >= 768:
        return 2  # whole block clamps to table[510]
    return 0      # in-band: needs the Toeplitz block


def _didx(t, u):
    return (_delta(t, u) + 256) // 128  # 0..7 for in-band blocks


def build_program():
    import os
    kdebug = os.environ.get("KDEBUG", "0") == "1"
    nc = bacc.Bacc()

    xqT = nc.declare_dram_parameter("xqT", [D, S], f16, isOutput=False)
    xkT = nc.declare_dram_parameter("xkT", [D, S], f16, isOutput=False)
    xvT = nc.declare_dram_parameter("xvT", [D, S], f16, isOutput=False)
    wq = nc.declare_dram_parameter("wq", [128, 4, 256], f16, isOutput=False)
    wk = nc.declare_dram_parameter("wk", [128, 4, 256], f16, isOutput=False)
    wv = nc.declare_dram_parameter("wv", [128, 4, 256], f16, isOutput=False)
    wo = nc.declare_dram_parameter("wo", [128, 2, 512], f16, isOutput=False)
    ebd = nc.declare_dram_parameter("eb", [128, 4, 8, 512], f16, isOutput=False)
    cbd = nc.declare_dram_parameter("cb", [128, 4, 3], f32, isOutput=False)
    outd = nc.declare_dram_parameter("out", [S, D], f32, isOutput=True)

    with tile.TileContext(nc) as tc:
        with (
            tc.tile_pool(name="sb", bufs=1) as pool,
            tc.tile_pool(name="xt", bufs=1) as xpool,
            tc.tile_pool(name="pt", bufs=6) as ppool,
            tc.tile_pool(name="cxs", bufs=1) as cpool,
            tc.tile_pool(name="obp", bufs=4) as opool,
            tc.tile_pool(name="scp", bufs=2, space="PSUM") as scp,
            tc.tile_pool(name="cxp", bufs=2, space="PSUM") as cxp,
            tc.tile_pool(name="msc", bufs=2, space="PSUM") as msc,
        ):
            # ---- persistent SBUF tiles -------------------------------------
            wq_sb = pool.tile([128, 4, 256], f16, tag="wq")
            wk_sb = pool.tile([128, 4, 256], f16, tag="wk")
            wv_sb = pool.tile([128, 4, 256], f16, tag="wv")
            wo_sb = pool.tile([128, 2, 512], f16, tag="wo")
            eb_sb = pool.tile([128, 4, 8, 512], f16, tag="eb")
            cb_sb = pool.tile([128, 4, 3], f32, tag="cb")
            qt_sb = pool.tile([128, 2, S], f16, tag="qt")
            kt_sb = pool.tile([128, 2, S], f16, tag="kt")
            v_sb = pool.tile([128, NKT, 4 * 65], f16, tag="v")
            onesb = pool.tile([128, 64], f16, tag="ones")
            warm = pool.tile([128, 16], f32, tag="warm")
            wtile = pool.tile([128, 512], f16, tag="wtile")

            # ---- DMAs, priority-ordered ------------------------------------
            # gpsimd issues the K path first in wave halves; gating schemes
            # and engine shuffles all measured worse (slow 0.4us/issue
            # sequencer trains + ring variance), so keep it simple.
            xks = [xpool.tile([128, S], f16, tag="xk", bufs=4, name=f"xk{ct}")
                   for ct in range(4)]
            nc.gpsimd.dma_start(wk_sb[:], wk[:])
            for half in range(2):
                for ct in range(4):
                    nc.gpsimd.dma_start(
                        xks[ct][:, half * 1024:(half + 1) * 1024],
                        xkT[ct * 128:(ct + 1) * 128,
                            half * 1024:(half + 1) * 1024])
            nc.gpsimd.dma_start(wo_sb[:], wo[:])
            # vector's user stream starts early (~7.3us); memset there so
            # the PE warm-up isn't gated on slow dma_start issues.
            nc.vector.memset(wtile[:], 0.0)
            # scalar queue: V path in halves, plus the tiny cb table.
            xvs = [xpool.tile([128, S], f16, tag="xv", bufs=4, name=f"xv{ct}")
                   for ct in range(4)]
            nc.scalar.dma_start(wv_sb[:], wv[:])
            nc.scalar.dma_start(cb_sb[:], cbd[:])
            for half in range(2):
                for ct in range(4):
                    nc.scalar.dma_start(
                        xvs[ct][:, half * 1024:(half + 1) * 1024],
                        xvT[ct * 128:(ct + 1) * 128,
                            half * 1024:(half + 1) * 1024])
            # sync queue: Q path in u-chunks (u0 first), then eb pieces in
            # first-use order (u0 in-band blocks need didx pairs 2,4,6).
            nc.sync.dma_start(wq_sb[:], wq[:])
            xqs = [xpool.tile([128, S], f16, tag="xq", bufs=4, name=f"xq{ct}")
                   for ct in range(4)]
            for u in range(NU):
                for ct in range(4):
                    nc.sync.dma_start(
                        xqs[ct][:, u * 512:(u + 1) * 512],
                        xqT[ct * 128:(ct + 1) * 128, u * 512:(u + 1) * 512])
            for d0 in (2, 4, 6, 0):
                nc.sync.dma_start(eb_sb[:, :, d0:d0 + 2, :],
                                  ebd[:, :, d0:d0 + 2, :])

            # PE p-state warm-up while the first DMAs land (bridges until
            # wk/xk wave 0 arrive so K-proj starts at full clock).
            wpsum = msc.tile([128, 512], f32, tag="ms", name="warmpm")
            for _ in range(12):
                nc.tensor.matmul(wpsum[0:16, :], lhsT=wtile[:, 0:16],
                                 rhs=wtile[:], start=True, stop=True)
            # stash the odd-head half of wo at partitions 0..63 (wtile is
            # free after the warm-up): lets the LAST slot's O-proj read the
            # un-shifted cxo tile directly, cutting the cxo SBUF-shift DMA
            # out of the tail's critical chain.
            nc.gpsimd.dma_start(wtile[0:64, :], wo_sb[64:128, 1, :])

            nc.vector.memset(onesb[:], 1.0)
            # preload the exp table while DMAs stream in
            nc.vector.memset(warm[:], 0.0)
            nc.scalar.activation(warm[:], warm[:], AF.Exp, bias=0.0, scale=1.0)

            # ---- phase A: K-proj, V-proj, Q-proj(u0) -----------------------
            # K-projection: 2 waves of (2 sc-chunks x 2 hp) using scp tiles.
            for wave in range(2):
                pks = {hp: scp.tile([128, 1024], f32, tag="sc", name=f"pk{wave}{hp}")
                       for hp in range(2)}
                for ct in range(4):
                    for hp in range(2):
                        for sc2 in range(2):
                            sc = wave * 2 + sc2
                            nc.tensor.matmul(
                                pks[hp][:, sc2 * 512:(sc2 + 1) * 512],
                                lhsT=wk_sb[:, ct, hp * 128:(hp + 1) * 128],
                                rhs=xks[ct][:, sc * 512:(sc + 1) * 512],
                                start=(ct == 0), stop=(ct == 3),
                            )
                for hp in range(2):
                    nc.vector.tensor_copy(
                        kt_sb[:, hp, wave * 1024:(wave + 1) * 1024], pks[hp][:])
            # V-projection: 2 waves of 8 s-tiles; ones column FIRST per head.
            for wave in range(2):
                pvs = [scp.tile([128, 1024], f32, tag="sc", name=f"pv{wave}{i}")
                       for i in range(2)]
                # st outer / ct inner: accumulation chains sharing a PSUM bank
                # must run sequentially (first_mm clears the bank's
                # has_written bits, so interleaved starts drop contributions)
                for st8 in range(8):
                    st = wave * 8 + st8
                    for ct in range(4):
                        nc.tensor.matmul(
                            pvs[st8 // 4][:, (st8 % 4) * 256:(st8 % 4) * 256 + 256],
                            lhsT=xvs[ct][:, st * 128:(st + 1) * 128],
                            rhs=wv_sb[:, ct, :],
                            start=(ct == 0), stop=(ct == 3),
                        )
                for st8 in range(8):
                    st = wave * 8 + st8
                    vslice = v_sb[:, st, :].rearrange("p (h x) -> p h x", x=65)
                    nc.vector.memset(vslice[:, :, 64:65], 1.0)
                    nc.vector.tensor_copy(
                        vslice[:, :, 0:64],
                        pvs[st8 // 4][:, (st8 % 4) * 256:(st8 % 4) * 256 + 256]
                        .rearrange("p (h x) -> p h x", x=64),
                    )

            def emit_qproj(u):
                for hp in range(2):
                    pq = msc.tile([128, 512], f32, tag="ms", name=f"pq{u}{hp}")
                    for ct in range(4):
                        nc.tensor.matmul(
                            pq[:],
                            lhsT=wq_sb[:, ct, hp * 128:(hp + 1) * 128],
                            rhs=xqs[ct][:, u * 512:(u + 1) * 512],
                            start=(ct == 0), stop=(ct == 3),
                        )
                    nc.vector.tensor_copy(qt_sb[:, hp, u * 512:(u + 1) * 512],
                                          pq[:])

            emit_qproj(0)

            def dump(dst_row, src_ap, nrows, ncols):
                t_ = opool.tile([nrows, 512], f32, tag="dbg", bufs=8,
                                name=f"dbg{dst_row}")
                nc.vector.tensor_copy(t_[:, 0:ncols], src_ap)
                nc.sync.dma_start(outd[dst_row:dst_row + nrows, 0:ncols],
                                  t_[:, 0:ncols])

            if kdebug:
                dump(256, qt_sb[:, 0, 0:512], 128, 512)
                dump(384, kt_sb[:, 0, 0:512], 128, 512)
                dump(512, v_sb[:, 0, 0:260], 128, 260)

            # ---- phase B: 8 slots (u, hp), norm/O-proj deferred one slot ---
            slots = [(u, hp) for u in range(NU) for hp in range(2)]
            prev = None          # (u, hp, ctxp pair) awaiting norm
            cxp2 = {}            # (u, hp) -> [128, 512] paired ctx tile
            cxo_last = [None]    # last slot's un-shifted odd-head cx tile
            pend_oproj = None    # u awaiting O-projection

            def emit_norm_pre(pv):
                # DVE part of the deferred norm chain for slot pv
                u0_, hp_, ctxp_ = pv
                outs = []
                for ah in range(2):
                    ctxf = cpool.tile([65, 512], f32, tag="ctxf", bufs=4)
                    nc.vector.tensor_copy(ctxf[:], ctxp_[ah][:])
                    if kdebug and (u0_, hp_) == (0, 0):
                        nc.sync.dma_start(
                            outd[640 + 65 * ah:640 + 65 * ah + 65, :], ctxf[:])
                    lp0 = cpool.tile([1, 512], f32, tag="lp0", bufs=2)
                    nc.gpsimd.dma_start(lp0[:], ctxf[64:65, :])
                    linv = cpool.tile([1, 512], f32, tag="linv", bufs=2)
                    nc.vector.reciprocal_approx_fast(linv[:], lp0[:])
                    linvb = cpool.tile([1, 512], f16, tag="linvb", bufs=2)
                    nc.vector.tensor_scalar_mul(linvb[:], linv[:], 256.0)
                    outs.append((ctxf, linvb))
                return outs

            def emit_norm_post(pv, outs):
                # PE broadcast + DVE normalize for slot pv; both heads land
                # in one [128, 512] tile (odd head DMA-shifted to parts
                # 64..127) so O-proj contracts 128-deep per chunk.
                u0_, hp_, _ = pv
                cxp = cpool.tile([128, 512], f16, tag="cx2", bufs=3,
                                 name=f"cxp{hp_}")
                cxp2[(u0_, hp_)] = cxp
                for ah in range(2):
                    ctxf, linvb = outs[ah]
                    bc = msc.tile([128, 512], f32, tag="ms", name=f"bc{hp_}{ah}")
                    nc.tensor.matmul(bc[0:64, :], lhsT=onesb[0:1, :],
                                     rhs=linvb[:], start=True, stop=True)
                    if ah == 0:
                        nc.vector.tensor_mul(cxp[0:64, :], bc[0:64, :],
                                             ctxf[0:64, :])
                    else:
                        cxo = cpool.tile([64, 512], f16, tag="cxo", bufs=2)
                        nc.vector.tensor_mul(cxo[:], bc[0:64, :],
                                             ctxf[0:64, :])
                        if (u0_, hp_) == (3, 1):
                            # last slot: O-proj reads cxo in place (3-chunk
                            # contraction) — skip the SBUF-shift DMA.
                            cxo_last[0] = cxo
                        else:
                            nc.gpsimd.dma_start(cxp[64:128, :], cxo[:])

            def emit_oproj_qs(u0_, qs):
                po = msc.tile([128, 512], f32, tag="ms", name=f"po{qs}")
                if u0_ == 3:
                    qsl = slice(qs * 128, (qs + 1) * 128)
                    nc.tensor.matmul(po[:], lhsT=cxp2[(3, 0)][:, qsl],
                                     rhs=wo_sb[:, 0, :],
                                     start=True, stop=False)
                    nc.tensor.matmul(po[:], lhsT=cxp2[(3, 1)][0:64, qsl],
                                     rhs=wo_sb[0:64, 1, :],
                                     start=False, stop=False)
                    nc.tensor.matmul(po[:], lhsT=cxo_last[0][:, qsl],
                                     rhs=wtile[0:64, :],
                                     start=False, stop=True)
                else:
                    for hp_ in range(2):
                        nc.tensor.matmul(
                            po[:],
                            lhsT=cxp2[(u0_, hp_)][:, qs * 128:(qs + 1) * 128],
                            rhs=wo_sb[:, hp_, :],
                            start=(hp_ == 0), stop=(hp_ == 1),
                        )
                ob = opool.tile([128, 512], f32, tag="ob")
                if u0_ == 3 and qs % 2 == 0:
                    nc.scalar.activation(ob[:], po[:], AF.Copy, bias=0.0,
                                         scale=1.0)
                else:
                    nc.vector.tensor_copy(ob[:], po[:])
                if not kdebug:
                    q_eng = (nc.sync, nc.gpsimd)[qs % 2] if u0_ == 3 else nc.sync
                    q_eng.dma_start(
                        outd[u0_ * 512 + qs * 128:
                             u0_ * 512 + (qs + 1) * 128, :],
                        ob[:],
                    )

            # Flat group stream across all slots; the AV matmuls for group i
            # are emitted at step i+1 so the PE never waits on ACT in-order.
            def slot_gorder(u):
                return sorted(range(NKT // 2),
                              key=lambda g: (_cls(2 * g, u) == 0, g))

            items = []
            for (u, hp) in slots:
                for gi, g in enumerate(slot_gorder(u)):
                    items.append((u, hp, gi, g))

            state = {}           # live per-slot state keyed by (u, hp)
            pend_av = None       # (slotkey, g, src, avstart)
            norm_outs = None

            def emit_av(slotkey, g, src):
                # ti-outer/ah-inner: consecutive matmuls hit different PSUM
                # tiles so the next ldweights overlaps the current drain.
                st_ = state[slotkey]
                u_, hp_ = slotkey
                for ti in range(2):
                    t = 2 * g + ti
                    for ah in range(2):
                        lh = 2 * hp_ + ah
                        vsl = v_sb[:, t, :].rearrange(
                            "p (h x) -> p h x", x=65)[:, lh, :]
                        st_["nav"][ah] += 1
                        nc.tensor.matmul(
                            st_["ctxp"][ah][:],
                            lhsT=vsl,
                            rhs=src[ah][:, ti * 512:(ti + 1) * 512],
                            start=(st_["nav"][ah] == 1),
                            stop=(st_["nav"][ah] == NKT),
                        )

            for (u, hp, gi, g) in items:
                slotkey = (u, hp)
                cls = _cls(2 * g, u)
                sct = [scp.tile([128, 1024], f32, tag="sc", name=f"sct{i}")
                       for i in range(2)]
                for ti in range(2):
                    t = 2 * g + ti
                    for ah in range(2):
                        nc.tensor.matmul(
                            sct[ah][:, ti * 512:(ti + 1) * 512],
                            lhsT=kt_sb[ah * 64:(ah + 1) * 64, hp,
                                       t * 128:(t + 1) * 128],
                            rhs=qt_sb[ah * 64:(ah + 1) * 64, hp,
                                      u * 512:(u + 1) * 512],
                            start=True, stop=True,
                            tile_position=(ah * 64, 0),
                        )
                src = []
                for ah in range(2):
                    lh = 2 * hp + ah
                    pt = ppool.tile([128, 1024], f16, tag="pt", bufs=6)
                    nc.scalar.activation(
                        pt[:], sct[ah][:], AF.Exp,
                        bias=cb_sb[:, lh, cls:cls + 1], scale=1.0,
                    )
                    if cls == 0:
                        sr = ppool.tile([128, 1024], f16, tag="pt2", bufs=4)
                        d0 = _didx(2 * g, u)
                        nc.vector.tensor_mul(
                            sr[:],
                            pt[:],
                            eb_sb[:, lh, d0:d0 + 2, :]
                            .rearrange("p a b -> p (a b)"),
                        )
                    else:
                        sr = pt
                    src.append(sr)

                if kdebug and (u, hp, gi) == (0, 0, 0):
                    dump(0, src[0][:, 0:512], 128, 512)
                    dump(128, src[1][:, 0:512], 128, 512)

                # AV for the previous group in the flat stream
                if pend_av is not None:
                    emit_av(*pend_av)
                if gi == 0:
                    # slot start (prev slot's last AV now emitted): deferred
                    # norm for prev slot, then this slot's accumulators.
                    norm_outs = emit_norm_pre(prev) if prev is not None else None
                    state[slotkey] = {
                        "ctxp": [cxp.tile([65, 512], f32, tag="cp",
                                          name=f"ctxp{ah}") for ah in range(2)],
                        "nav": [0, 0],
                    }
                pend_av = (slotkey, g, src)

                # deferred work rides the ACT-bound slack
                if gi == 1 and norm_outs is not None:
                    emit_norm_post(prev, norm_outs)
                    if prev[1] == 1:       # hp==1 slot completed unit u-1
                        pend_oproj = prev[0]
                    norm_outs = None
                if gi in (3, 4, 5, 6) and pend_oproj is not None:
                    emit_oproj_qs(pend_oproj, gi - 3)
                    if gi == 6:
                        pend_oproj = None
                if gi == 7 and hp == 0 and u < 3:
                    emit_qproj(u + 1)
                if gi == 7:
                    prev = (u, hp, state[slotkey]["ctxp"])

            emit_av(*pend_av)

            # ---- tail: last slot's norm + O-proj ---------------------------
            norm_outs = emit_norm_pre(prev)
            emit_norm_post(prev, norm_outs)
            for qs in range(4):
                emit_oproj_qs(3, qs)
    nc.compile()
    return nc


_PROGRAM = None


def _get_program():
    global _PROGRAM
    if _PROGRAM is None:
        _PROGRAM = build_program()
    return _PROGRAM


# index table for the in-band Toeplitz bias blocks, shared across heads
_IDX = None


def _idx_table():
    global _IDX
    if _IDX is None:
        p = np.arange(128)[:, None]
        f = np.arange(512)[None, :]
        blocks = []
        for didx in range(8):
            delta = didx * 128 - 256
            blocks.append(np.clip(delta + p - f + 255, 0, 510))
        _IDX = np.stack(blocks, axis=0)  # [8, 128, 512]
    return _IDX


def kernel(**inputs):
    query = np.asarray(inputs["query"], dtype=np.float32)
    key = np.asarray(inputs["key"], dtype=np.float32)
    value = np.asarray(inputs["value"], dtype=np.float32)
    mask = np.asarray(inputs["mask"])
    Wq = np.asarray(inputs["Wq"], dtype=np.float32)
    Wk = np.asarray(inputs["Wk"], dtype=np.float32)
    Wv = np.asarray(inputs["Wv"], dtype=np.float32)
    Wo = np.asarray(inputs["Wo"], dtype=np.float32)
    bo = np.asarray(inputs["bo"], dtype=np.float32)
    rel_bias = np.asarray(inputs["rel_bias"], dtype=np.float32)

    if not np.all(mask != 0):
        raise NotImplementedError("kernel assumes an all-ones attention mask")

    nc = _get_program()
    idx = _idx_table()
    scale = np.float32(1.0 / np.sqrt(DK))

    in_maps = []
    for c in range(NCORES):
        b = c // 2
        hbase = (c % 2) * 4
        rows = slice(hbase * 64, (hbase + 4) * 64)

        wq_arr = np.ascontiguousarray(
            (Wq[rows, :] * scale).T.reshape(4, 128, 256).swapaxes(0, 1))
        wk_arr = np.ascontiguousarray(
            Wk[rows, :].T.reshape(4, 128, 256).swapaxes(0, 1))
        wv_arr = np.ascontiguousarray(
            Wv[rows, :].T.reshape(4, 128, 256).swapaxes(0, 1))

        wo_arr = np.empty((128, 2, 512), dtype=np.float32)
        eb_arr = np.empty((128, 4, 8, 512), dtype=np.float16)
        cb_arr = np.zeros((128, 4, 3), dtype=np.float32)
        for lh in range(4):
            g = hbase + lh
            hp, ah = divmod(lh, 2)
            wo_arr[ah * 64:(ah + 1) * 64, hp, :] = (
                Wo[:, g * 64:(g + 1) * 64].T * (1.0 / 256.0))
            tbl = rel_bias[g]
            eb_arr[:, lh, :, :] = np.exp(tbl)[idx].transpose(1, 0, 2)
            cb_arr[:, lh, 1] = tbl[0]
            cb_arr[:, lh, 2] = tbl[510]

        bf = np.float16
        in_maps.append({
            "xqT": np.ascontiguousarray(query[b].T).astype(bf),
            "xkT": np.ascontiguousarray(key[b].T).astype(bf),
            "xvT": np.ascontiguousarray(value[b].T).astype(bf),
            "wq": wq_arr.astype(bf), "wk": wk_arr.astype(bf),
            "wv": wv_arr.astype(bf), "wo": wo_arr.astype(bf),
            "eb": eb_arr, "cb": cb_arr,
        })

    res = run_bass_kernel_spmd(nc, in_maps, list(range(NCORES)), trace=False)

    out = np.zeros((B, S, D), dtype=np.float32)
    for c in range(NCORES):
        out[c // 2] += res.results[c]["out"]
    out += bo[None, None, :]
    return out



# revision 49
# speedup vs baseline: 1.1854x; 1.0047x over previous
"""Trainium2 Bass kernel for nn_MultiHeadAttention_34162169872901 — v3.

MultiHeadAttention (B=4, S=2048, d_model=512, 8 heads, d_k=64) with a
relative-position bias table (511 entries, clamp +-255) and an all-ones mask.

Sharding (8 NeuronCores): core c handles batch b = c//2 and 4 of the 8 heads
(c%2 selects the head half); host sums the two partial outputs per batch.

v3 changes vs v2 (222.8us):
  - Input DMAs issued from the tensor/scalar sequencers (their preamble is
    ~6us shorter than sync's) and xk/xv split in wave-halves, so K-projection
    starts ~7us earlier and phase-A DMA stalls shrink.
  - PE p-state warm-up matmuls during the initial DMA wait.
  - O-projection packed to 128-deep contraction: the odd head's normalized
    ctx is DMA-shifted into partitions 64..127 of a paired cx tile, halving
    O-proj matmul count.
  - QK matmuls emitted ah-major so PE tile_position switches halve and the
    first exp of each group starts one matmul earlier.
"""

import sys
import types

import numpy as np

B = 4
S = 2048
D = 512
NHEAD = 8
DK = 64
NCORES = 8
MAX_REL = 255
NKT = S // 128   # 16 k-tiles
NU = S // 512    # 4 q-units


def _install_axon_hooks():
    try:
        import antenv
    except ImportError:
        return
    try:
        from antenv.axon_hooks import get_axon_ntff_profile_hook  # noqa: F401
        return
    except ImportError:
        pass
    hook = None
    try:
        from trn_agent_boot.trn_boot import _ntff_profile_via_ctypes
        hook = _ntff_profile_via_ctypes("/opt/axon/libaxon_pjrt.so")
    except Exception:
        hook = None
    m = types.ModuleType("antenv.axon_hooks")
    m.get_axon_ntff_profile_hook = lambda: hook
    m.set_axon_ntff_profile_hook = lambda h: None
    sys.modules["antenv.axon_hooks"] = m
    antenv.axon_hooks = m


_install_axon_hooks()

import concourse.bass as bass  # noqa: E402
import concourse.bacc as bacc  # noqa: E402
import concourse.mybir as mybir  # noqa: E402
from concourse import tile  # noqa: E402
from concourse.bass_utils import run_bass_kernel_spmd  # noqa: E402
from concourse.vector_clock import ScopedClock as _ScopedClock  # noqa: E402

f32 = mybir.dt.float32
f32r = mybir.dt.float32r
f16 = mybir.dt.float16
AF = mybir.ActivationFunctionType


def _patched_drain_and_barrier(self, tick_clock, wait_clock):
    # walrus in this container rejects >2 sem waits on one instruction; emit
    # the tail-drain waits as standalone wait instructions instead.
    nc = self.nc
    dummy = mybir.InstNoOp(name="drain-wait-probe", engine=mybir.EngineType.SP)
    wait_clock.add_sem_waits(dummy, _ScopedClock({None: tick_clock.global_clock}))
    handles = {h.name: h for h in self.sems.allocated().values()}
    si = dummy.sync_info
    for w in (si.on_wait if si is not None else []):
        nc.sync.wait_ge(handles[w.ant_name], w.wait_value)
    nc.sync.drain()
    nc.all_engine_barrier()
    popped = nc._tile_sem_poison_stack.pop()
    assert popped is self._sem_poison
    nc.clear_and_free_semaphores(list(self.sems.allocated().values()))
    nc.all_engine_barrier()


tile.TileContext._drain_and_barrier = _patched_drain_and_barrier


def _delta(t, u):
    return 128 * t - 512 * u


def _cls(t, u):
    d = _delta(t, u)
    if d <= -384:
        return 1  # whole block clamps to table[0]
    if d >= 768:
        return 2  # whole block clamps to table[510]
    return 0      # in-band: needs the Toeplitz block


def _didx(t, u):
    return (_delta(t, u) + 256) // 128  # 0..7 for in-band blocks


def build_program():
    import os
    kdebug = os.environ.get("KDEBUG", "0") == "1"
    nc = bacc.Bacc()

    xqT = nc.declare_dram_parameter("xqT", [D, S], f16, isOutput=False)
    xkT = nc.declare_dram_parameter("xkT", [D, S], f16, isOutput=False)
    xvT = nc.declare_dram_parameter("xvT", [D, S], f16, isOutput=False)
    wq = nc.declare_dram_parameter("wq", [128, 4, 256], f16, isOutput=False)
    wk = nc.declare_dram_parameter("wk", [128, 4, 256], f16, isOutput=False)
    wv = nc.declare_dram_parameter("wv", [128, 4, 256], f16, isOutput=False)
    wo = nc.declare_dram_parameter("wo", [128, 2, 512], f16, isOutput=False)
    ebd = nc.declare_dram_parameter("eb", [128, 4, 8, 512], f16, isOutput=False)
    cbd = nc.declare_dram_parameter("cb", [128, 4, 3], f32, isOutput=False)
    outd = nc.declare_dram_parameter("out", [S, D], f32, isOutput=True)

    with tile.TileContext(nc) as tc:
        with (
            tc.tile_pool(name="sb", bufs=1) as pool,
            tc.tile_pool(name="xt", bufs=1) as xpool,
            tc.tile_pool(name="pt", bufs=6) as ppool,
            tc.tile_pool(name="cxs", bufs=1) as cpool,
            tc.tile_pool(name="obp", bufs=4) as opool,
            tc.tile_pool(name="scp", bufs=2, space="PSUM") as scp,
            tc.tile_pool(name="cxp", bufs=2, space="PSUM") as cxp,
            tc.tile_pool(name="msc", bufs=2, space="PSUM") as msc,
        ):
            # ---- persistent SBUF tiles -------------------------------------
            wq_sb = pool.tile([128, 4, 256], f16, tag="wq")
            wk_sb = pool.tile([128, 4, 256], f16, tag="wk")
            wv_sb = pool.tile([128, 4, 256], f16, tag="wv")
            wo_sb = pool.tile([128, 2, 512], f16, tag="wo")
            eb_sb = pool.tile([128, 4, 8, 512], f16, tag="eb")
            cb_sb = pool.tile([128, 4, 3], f32, tag="cb")
            qt_sb = pool.tile([128, 2, S], f16, tag="qt")
            kt_sb = pool.tile([128, 2, S], f16, tag="kt")
            v_sb = pool.tile([128, NKT, 4 * 65], f16, tag="v")
            onesb = pool.tile([128, 64], f16, tag="ones")
            warm = pool.tile([128, 16], f32, tag="warm")
            wtile = pool.tile([128, 512], f16, tag="wtile")

            # ---- DMAs, priority-ordered ------------------------------------
            # gpsimd issues the K path first in wave halves; gating schemes
            # and engine shuffles all measured worse (slow 0.4us/issue
            # sequencer trains + ring variance), so keep it simple.
            xks = [xpool.tile([128, S], f16, tag="xk", bufs=4, name=f"xk{ct}")
                   for ct in range(4)]
            nc.gpsimd.dma_start(wk_sb[:], wk[:])
            for half in range(2):
                for ct in range(4):
                    nc.gpsimd.dma_start(
                        xks[ct][:, half * 1024:(half + 1) * 1024],
                        xkT[ct * 128:(ct + 1) * 128,
                            half * 1024:(half + 1) * 1024])
            nc.gpsimd.dma_start(wo_sb[:], wo[:])
            # vector's user stream starts early (~7.3us); memset there so
            # the PE warm-up isn't gated on slow dma_start issues.
            nc.vector.memset(wtile[:], 0.0)
            # scalar queue: V path in halves, plus the tiny cb table.
            xvs = [xpool.tile([128, S], f16, tag="xv", bufs=4, name=f"xv{ct}")
                   for ct in range(4)]
            nc.scalar.dma_start(wv_sb[:], wv[:])
            nc.scalar.dma_start(cb_sb[:], cbd[:])
            for half in range(2):
                for ct in range(4):
                    nc.scalar.dma_start(
                        xvs[ct][:, half * 1024:(half + 1) * 1024],
                        xvT[ct * 128:(ct + 1) * 128,
                            half * 1024:(half + 1) * 1024])
            # sync queue: Q path in u-chunks (u0 first), then eb pieces in
            # first-use order (u0 in-band blocks need didx pairs 2,4,6).
            nc.sync.dma_start(wq_sb[:], wq[:])
            xqs = [xpool.tile([128, S], f16, tag="xq", bufs=4, name=f"xq{ct}")
                   for ct in range(4)]
            for u in range(NU):
                for ct in range(4):
                    nc.sync.dma_start(
                        xqs[ct][:, u * 512:(u + 1) * 512],
                        xqT[ct * 128:(ct + 1) * 128, u * 512:(u + 1) * 512])
            for d0 in (2, 4, 6, 0):
                nc.sync.dma_start(eb_sb[:, :, d0:d0 + 2, :],
                                  ebd[:, :, d0:d0 + 2, :])

            # PE p-state warm-up while the first DMAs land (bridges until
            # wk/xk wave 0 arrive so K-proj starts at full clock).
            wpsum = msc.tile([128, 512], f32, tag="ms", name="warmpm")
            for _ in range(12):
                nc.tensor.matmul(wpsum[0:16, :], lhsT=wtile[:, 0:16],
                                 rhs=wtile[:], start=True, stop=True)
            # stash the odd-head half of wo at partitions 0..63 (wtile is
            # free after the warm-up): lets the LAST slot's O-proj read the
            # un-shifted cxo tile directly, cutting the cxo SBUF-shift DMA
            # out of the tail's critical chain.
            nc.gpsimd.dma_start(wtile[0:64, :], wo_sb[64:128, 1, :])

            nc.vector.memset(onesb[:], 1.0)
            # preload the exp table while DMAs stream in
            nc.vector.memset(warm[:], 0.0)
            nc.scalar.activation(warm[:], warm[:], AF.Exp, bias=0.0, scale=1.0)

            # ---- phase A: K-proj, V-proj, Q-proj(u0) -----------------------
            # K-projection: 2 waves of (2 sc-chunks x 2 hp) using scp tiles.
            for wave in range(2):
                pks = {hp: scp.tile([128, 1024], f32, tag="sc", name=f"pk{wave}{hp}")
                       for hp in range(2)}
                for ct in range(4):
                    for hp in range(2):
                        for sc2 in range(2):
                            sc = wave * 2 + sc2
                            nc.tensor.matmul(
                                pks[hp][:, sc2 * 512:(sc2 + 1) * 512],
                                lhsT=wk_sb[:, ct, hp * 128:(hp + 1) * 128],
                                rhs=xks[ct][:, sc * 512:(sc + 1) * 512],
                                start=(ct == 0), stop=(ct == 3),
                            )
                for hp in range(2):
                    nc.vector.tensor_copy(
                        kt_sb[:, hp, wave * 1024:(wave + 1) * 1024], pks[hp][:])
            # V-projection: 2 waves of 8 s-tiles; ones column FIRST per head.
            for wave in range(2):
                pvs = [scp.tile([128, 1024], f32, tag="sc", name=f"pv{wave}{i}")
                       for i in range(2)]
                # st outer / ct inner: accumulation chains sharing a PSUM bank
                # must run sequentially (first_mm clears the bank's
                # has_written bits, so interleaved starts drop contributions)
                for st8 in range(8):
                    st = wave * 8 + st8
                    for ct in range(4):
                        nc.tensor.matmul(
                            pvs[st8 // 4][:, (st8 % 4) * 256:(st8 % 4) * 256 + 256],
                            lhsT=xvs[ct][:, st * 128:(st + 1) * 128],
                            rhs=wv_sb[:, ct, :],
                            start=(ct == 0), stop=(ct == 3),
                        )
                for st8 in range(8):
                    st = wave * 8 + st8
                    vslice = v_sb[:, st, :].rearrange("p (h x) -> p h x", x=65)
                    nc.vector.memset(vslice[:, :, 64:65], 1.0)
                    nc.vector.tensor_copy(
                        vslice[:, :, 0:64],
                        pvs[st8 // 4][:, (st8 % 4) * 256:(st8 % 4) * 256 + 256]
                        .rearrange("p (h x) -> p h x", x=64),
                    )

            def emit_qproj(u):
                for hp in range(2):
                    pq = msc.tile([128, 512], f32, tag="ms", name=f"pq{u}{hp}")
                    for ct in range(4):
                        nc.tensor.matmul(
                            pq[:],
                            lhsT=wq_sb[:, ct, hp * 128:(hp + 1) * 128],
                            rhs=xqs[ct][:, u * 512:(u + 1) * 512],
                            start=(ct == 0), stop=(ct == 3),
                        )
                    nc.vector.tensor_copy(qt_sb[:, hp, u * 512:(u + 1) * 512],
                                          pq[:])

            emit_qproj(0)

            def dump(dst_row, src_ap, nrows, ncols):
                t_ = opool.tile([nrows, 512], f32, tag="dbg", bufs=8,
                                name=f"dbg{dst_row}")
                nc.vector.tensor_copy(t_[:, 0:ncols], src_ap)
                nc.sync.dma_start(outd[dst_row:dst_row + nrows, 0:ncols],
                                  t_[:, 0:ncols])

            if kdebug:
                dump(256, qt_sb[:, 0, 0:512], 128, 512)
                dump(384, kt_sb[:, 0, 0:512], 128, 512)
                dump(512, v_sb[:, 0, 0:260], 128, 260)

            # ---- phase B: 8 slots (u, hp), norm/O-proj deferred one slot ---
            slots = [(u, hp) for u in range(NU) for hp in range(2)]
            prev = None          # (u, hp, ctxp pair) awaiting norm
            cxp2 = {}            # (u, hp) -> [128, 512] paired ctx tile
            cxo_last = [None]    # last slot's un-shifted odd-head cx tile
            pend_oproj = None    # u awaiting O-projection

            def emit_norm_pre(pv):
                # DVE part of the deferred norm chain for slot pv
                u0_, hp_, ctxp_ = pv
                outs = []
                for ah in range(2):
                    ctxf = cpool.tile([65, 512], f32, tag="ctxf", bufs=4)
                    nc.vector.tensor_copy(ctxf[:], ctxp_[ah][:])
                    if kdebug and (u0_, hp_) == (0, 0):
                        nc.sync.dma_start(
                            outd[640 + 65 * ah:640 + 65 * ah + 65, :], ctxf[:])
                    # dead alloc keeps the cpool address map unchanged now
                    # that the lp0 SBUF hop is gone (reciprocal reads the
                    # PSUM l-row directly; custom DVE ops need base
                    # partition 0, so run over partitions 0..65 — cost is
                    # free-size only).
                    _lp0_pad = cpool.tile([1, 512], f32, tag="lp0",
                                          bufs=2, name="lp0")
                    linv = cpool.tile([65, 512], f32, tag="linv", bufs=2)
                    nc.vector.reciprocal_approx_fast(linv[:], ctxp_[ah][:])
                    linvb = cpool.tile([65, 512], f16, tag="linvb", bufs=2)
                    nc.vector.tensor_scalar_mul(linvb[64:65, :],
                                                linv[64:65, :], 256.0)
                    outs.append((ctxf, linvb))
                return outs

            def emit_norm_post(pv, outs):
                # PE broadcast + DVE normalize for slot pv; both heads land
                # in one [128, 512] tile (odd head DMA-shifted to parts
                # 64..127) so O-proj contracts 128-deep per chunk.
                u0_, hp_, _ = pv
                cxp = cpool.tile([128, 512], f16, tag="cx2", bufs=3,
                                 name=f"cxp{hp_}")
                cxp2[(u0_, hp_)] = cxp
                for ah in range(2):
                    ctxf, linvb = outs[ah]
                    bc = msc.tile([128, 512], f32, tag="ms", name=f"bc{hp_}{ah}")
                    nc.tensor.matmul(bc[0:64, :], lhsT=onesb[64:65, :],
                                     rhs=linvb[64:65, :], start=True,
                                     stop=True)
                    if ah == 0:
                        nc.vector.tensor_mul(cxp[0:64, :], bc[0:64, :],
                                             ctxf[0:64, :])
                    else:
                        cxo = cpool.tile([64, 512], f16, tag="cxo", bufs=2)
                        nc.vector.tensor_mul(cxo[:], bc[0:64, :],
                                             ctxf[0:64, :])
                        if (u0_, hp_) == (3, 1):
                            # last slot: O-proj reads cxo in place (3-chunk
                            # contraction) — skip the SBUF-shift DMA.
                            cxo_last[0] = cxo
                        else:
                            nc.gpsimd.dma_start(cxp[64:128, :], cxo[:])

            def emit_oproj_qs(u0_, qs):
                po = msc.tile([128, 512], f32, tag="ms", name=f"po{qs}")
                if u0_ == 3:
                    qsl = slice(qs * 128, (qs + 1) * 128)
                    nc.tensor.matmul(po[:], lhsT=cxp2[(3, 0)][:, qsl],
                                     rhs=wo_sb[:, 0, :],
                                     start=True, stop=False)
                    nc.tensor.matmul(po[:], lhsT=cxp2[(3, 1)][0:64, qsl],
                                     rhs=wo_sb[0:64, 1, :],
                                     start=False, stop=False)
                    nc.tensor.matmul(po[:], lhsT=cxo_last[0][:, qsl],
                                     rhs=wtile[0:64, :],
                                     start=False, stop=True)
                else:
                    for hp_ in range(2):
                        nc.tensor.matmul(
                            po[:],
                            lhsT=cxp2[(u0_, hp_)][:, qs * 128:(qs + 1) * 128],
                            rhs=wo_sb[:, hp_, :],
                            start=(hp_ == 0), stop=(hp_ == 1),
                        )
                ob = opool.tile([128, 512], f32, tag="ob")
                if u0_ == 3 and qs % 2 == 0:
                    nc.scalar.activation(ob[:], po[:], AF.Copy, bias=0.0,
                                         scale=1.0)
                else:
                    nc.vector.tensor_copy(ob[:], po[:])
                if not kdebug:
                    q_eng = (nc.sync, nc.gpsimd)[qs % 2] if u0_ == 3 else nc.sync
                    q_eng.dma_start(
                        outd[u0_ * 512 + qs * 128:
                             u0_ * 512 + (qs + 1) * 128, :],
                        ob[:],
                    )

            # Flat group stream across all slots; the AV matmuls for group i
            # are emitted at step i+1 so the PE never waits on ACT in-order.
            def slot_gorder(u):
                return sorted(range(NKT // 2),
                              key=lambda g: (_cls(2 * g, u) == 0, g))

            items = []
            for (u, hp) in slots:
                for gi, g in enumerate(slot_gorder(u)):
                    items.append((u, hp, gi, g))

            state = {}           # live per-slot state keyed by (u, hp)
            pend_av = None       # (slotkey, g, src, avstart)
            norm_outs = None

            def emit_av(slotkey, g, src):
                # ti-outer/ah-inner: consecutive matmuls hit different PSUM
                # tiles so the next ldweights overlaps the current drain.
                st_ = state[slotkey]
                u_, hp_ = slotkey
                for ti in range(2):
                    t = 2 * g + ti
                    for ah in range(2):
                        lh = 2 * hp_ + ah
                        vsl = v_sb[:, t, :].rearrange(
                            "p (h x) -> p h x", x=65)[:, lh, :]
                        st_["nav"][ah] += 1
                        nc.tensor.matmul(
                            st_["ctxp"][ah][:],
                            lhsT=vsl,
                            rhs=src[ah][:, ti * 512:(ti + 1) * 512],
                            start=(st_["nav"][ah] == 1),
                            stop=(st_["nav"][ah] == NKT),
                        )

            for (u, hp, gi, g) in items:
                slotkey = (u, hp)
                cls = _cls(2 * g, u)
                sct = [scp.tile([128, 1024], f32, tag="sc", name=f"sct{i}")
                       for i in range(2)]
                for ti in range(2):
                    t = 2 * g + ti
                    for ah in range(2):
                        nc.tensor.matmul(
                            sct[ah][:, ti * 512:(ti + 1) * 512],
                            lhsT=kt_sb[ah * 64:(ah + 1) * 64, hp,
                                       t * 128:(t + 1) * 128],
                            rhs=qt_sb[ah * 64:(ah + 1) * 64, hp,
                                      u * 512:(u + 1) * 512],
                            start=True, stop=True,
                            tile_position=(ah * 64, 0),
                        )
                src = []
                for ah in range(2):
                    lh = 2 * hp + ah
                    pt = ppool.tile([128, 1024], f16, tag="pt", bufs=6)
                    nc.scalar.activation(
                        pt[:], sct[ah][:], AF.Exp,
                        bias=cb_sb[:, lh, cls:cls + 1], scale=1.0,
                    )
                    if cls == 0:
                        sr = ppool.tile([128, 1024], f16, tag="pt2", bufs=4)
                        d0 = _didx(2 * g, u)
                        nc.vector.tensor_mul(
                            sr[:],
                            pt[:],
                            eb_sb[:, lh, d0:d0 + 2, :]
                            .rearrange("p a b -> p (a b)"),
                        )
                    else:
                        sr = pt
                    src.append(sr)

                if kdebug and (u, hp, gi) == (0, 0, 0):
                    dump(0, src[0][:, 0:512], 128, 512)
                    dump(128, src[1][:, 0:512], 128, 512)

                # AV for the previous group in the flat stream
                if pend_av is not None:
                    emit_av(*pend_av)
                if gi == 0:
                    # slot start (prev slot's last AV now emitted): deferred
                    # norm for prev slot, then this slot's accumulators.
                    norm_outs = emit_norm_pre(prev) if prev is not None else None
                    state[slotkey] = {
                        "ctxp": [cxp.tile([65, 512], f32, tag="cp",
                                          name=f"ctxp{ah}") for ah in range(2)],
                        "nav": [0, 0],
                    }
                pend_av = (slotkey, g, src)

                # deferred work rides the ACT-bound slack
                if gi == 1 and norm_outs is not None:
                    emit_norm_post(prev, norm_outs)
                    if prev[1] == 1:       # hp==1 slot completed unit u-1
                        pend_oproj = prev[0]
                    norm_outs = None
                if gi in (3, 4, 5, 6) and pend_oproj is not None:
                    emit_oproj_qs(pend_oproj, gi - 3)
                    if gi == 6:
                        pend_oproj = None
                if gi == 7 and hp == 0 and u < 3:
                    emit_qproj(u + 1)
                if gi == 7:
                    prev = (u, hp, state[slotkey]["ctxp"])

            emit_av(*pend_av)

            # ---- tail: last slot's norm + O-proj ---------------------------
            norm_outs = emit_norm_pre(prev)
            emit_norm_post(prev, norm_outs)
            for qs in range(4):
                emit_oproj_qs(3, qs)
    nc.compile()
    return nc


_PROGRAM = None


def _get_program():
    global _PROGRAM
    if _PROGRAM is None:
        _PROGRAM = build_program()
    return _PROGRAM


# index table for the in-band Toeplitz bias blocks, shared across heads
_IDX = None


def _idx_table():
    global _IDX
    if _IDX is None:
        p = np.arange(128)[:, None]
        f = np.arange(512)[None, :]
        blocks = []
        for didx in range(8):
            delta = didx * 128 - 256
            blocks.append(np.clip(delta + p - f + 255, 0, 510))
        _IDX = np.stack(blocks, axis=0)  # [8, 128, 512]
    return _IDX


def kernel(**inputs):
    query = np.asarray(inputs["query"], dtype=np.float32)
    key = np.asarray(inputs["key"], dtype=np.float32)
    value = np.asarray(inputs["value"], dtype=np.float32)
    mask = np.asarray(inputs["mask"])
    Wq = np.asarray(inputs["Wq"], dtype=np.float32)
    Wk = np.asarray(inputs["Wk"], dtype=np.float32)
    Wv = np.asarray(inputs["Wv"], dtype=np.float32)
    Wo = np.asarray(inputs["Wo"], dtype=np.float32)
    bo = np.asarray(inputs["bo"], dtype=np.float32)
    rel_bias = np.asarray(inputs["rel_bias"], dtype=np.float32)

    if not np.all(mask != 0):
        raise NotImplementedError("kernel assumes an all-ones attention mask")

    nc = _get_program()
    idx = _idx_table()
    scale = np.float32(1.0 / np.sqrt(DK))

    in_maps = []
    for c in range(NCORES):
        b = c // 2
        hbase = (c % 2) * 4
        rows = slice(hbase * 64, (hbase + 4) * 64)

        wq_arr = np.ascontiguousarray(
            (Wq[rows, :] * scale).T.reshape(4, 128, 256).swapaxes(0, 1))
        wk_arr = np.ascontiguousarray(
            Wk[rows, :].T.reshape(4, 128, 256).swapaxes(0, 1))
        wv_arr = np.ascontiguousarray(
            Wv[rows, :].T.reshape(4, 128, 256).swapaxes(0, 1))

        wo_arr = np.empty((128, 2, 512), dtype=np.float32)
        eb_arr = np.empty((128, 4, 8, 512), dtype=np.float16)
        cb_arr = np.zeros((128, 4, 3), dtype=np.float32)
        for lh in range(4):
            g = hbase + lh
            hp, ah = divmod(lh, 2)
            wo_arr[ah * 64:(ah + 1) * 64, hp, :] = (
                Wo[:, g * 64:(g + 1) * 64].T * (1.0 / 256.0))
            tbl = rel_bias[g]
            eb_arr[:, lh, :, :] = np.exp(tbl)[idx].transpose(1, 0, 2)
            cb_arr[:, lh, 1] = tbl[0]
            cb_arr[:, lh, 2] = tbl[510]

        bf = np.float16
        in_maps.append({
            "xqT": np.ascontiguousarray(query[b].T).astype(bf),
            "xkT": np.ascontiguousarray(key[b].T).astype(bf),
            "xvT": np.ascontiguousarray(value[b].T).astype(bf),
            "wq": wq_arr.astype(bf), "wk": wk_arr.astype(bf),
            "wv": wv_arr.astype(bf), "wo": wo_arr.astype(bf),
            "eb": eb_arr, "cb": cb_arr,
        })

    res = run_bass_kernel_spmd(nc, in_maps, list(range(NCORES)), trace=False)

    out = np.zeros((B, S, D), dtype=np.float32)
    for c in range(NCORES):
        out[c // 2] += res.results[c]["out"]
    out += bo[None, None, :]
    return out

